# revision 24
# baseline (speedup 1.0000x reference)
"""Trainium2 Bass kernel for nn_DualOrganism (gnn_message_passing).

Strategy (validated numerically in a numpy twin model):
- 8-way data-parallel over cells: core c owns cells [1024c, 1024c+1024).
- All-pairs masks via exact-integer d2 computed with bf16-exact split rows
  on the PE (K=8 matmul), turned into S = sign(25.5 - d2) in {-1,+1} on ACT.
- Neighbor aggregation as S @ [P | a*P] bf16 matmuls (hi/lo split of P keeps
  ~17 mantissa bits); same/other-organism aggregates recovered algebraically:
      sameagg = (G + S@P)/4 + a_i*(G_a + S@(aP))/4 - P_i
      other_fi = (U_fi - a_i*U_{a fi})/4
  Counts (cnt, same_mass, same_fi, other_fi) are exact integers.
- Nearest same-org FI cell: argmin over exact integer keys
  key = 32*d2 + 32*BIG*(1-cand) + (j mod 32), segment-min reduction with
  first-index tie-breaking identical to jnp.argmin.
- Force field: host-deduped (last-writer-wins) indirect-DMA scatter to a DRAM
  grid, separable 15-tap blur as banded matmuls field = B @ egrid @ B,
  indirect-DMA gather of potentials.
- Per-cell MLPs computed in transposed layout for both organisms + select.

Host side only marshals inputs (layout/packing/index dedup) and reassembles
the outputs; all model math runs on the NeuronCores.
"""

import os
import sys

if "/opt/trn_rl_repo" not in sys.path:
    sys.path.insert(0, "/opt/trn_rl_repo")

import numpy as np
import ml_dtypes

import concourse.bass as bass
import concourse.mybir as mybir
import concourse.tile as tile
from concourse import bacc
from concourse.bass import IndirectOffsetOnAxis

F32 = mybir.dt.float32
BF16 = mybir.dt.bfloat16
I32 = mybir.dt.int32
AX = mybir.AxisListType
OP = mybir.AluOpType
ACTF = mybir.ActivationFunctionType

GRID = 256
N = 8192
D = 32
H = 64
KM = 15
SIGMA = 1.5
NCORES = 8
M = N // NCORES          # 1024 cells per core
NBLK = M // 128          # 8 i-blocks per core
NCH = N // 128           # 64 j-chunks
BIG = 1 << 17
HUGE = float(1 << 24)
PC = 131                 # P columns (pselo 64 | pseli 64 | is_mass | is_fi | 1)
PC2 = 2 * PC             # with a*P appended

bf = ml_dtypes.bfloat16


# ----------------------------------------------------------------------------
# host-side input marshalling (numpy only)
# ----------------------------------------------------------------------------

def _split_hi_lo(xf32):
    hi = xf32.astype(bf)
    lo = (xf32 - hi.astype(np.float32)).astype(bf)
    return hi, lo


def _band_matrix():
    ax = (np.arange(KM, dtype=np.float32) - (KM - 1) / 2.0).astype(np.float32)
    k1 = np.exp(-(ax * ax) / np.float32(2.0 * SIGMA * SIGMA)).astype(np.float32)
    k1n = (k1 / k1.sum()).astype(np.float32)
    B = np.zeros((GRID, GRID), np.float32)
    for u in range(GRID):
        lo = max(0, u - 7)
        hi = min(GRID, u + 8)
        B[u, lo:hi] = k1n[lo - u + 7:hi - u + 7]
    return B


def _dedup_scatter_offsets(x, y):
    offs = (y.astype(np.int64) * GRID + x.astype(np.int64)).astype(np.int64)
    out = np.full(N, GRID * GRID, np.int32)          # trash slot
    # last writer wins: np.unique keeps first occurrence -> reverse
    rev = offs[::-1]
    _, first_idx = np.unique(rev, return_index=True)
    winners = (N - 1) - first_idx
    out[winners] = offs[winners].astype(np.int32)
    return out


def host_prep(inputs):
    """Build all device input arrays. Returns (shared dict, per-core list)."""
    pos = np.asarray(inputs["positions"])
    states = np.asarray(inputs["states"], np.float32)
    roles = np.asarray(inputs["roles"])
    energies = np.asarray(inputs["energies"], np.float32)
    org = np.asarray(inputs["org_ids"])
    rand = np.asarray(inputs["rand_switch"], np.float32)

    x = pos[:, 0].astype(np.int64)
    y = pos[:, 1].astype(np.int64)
    sq = x * x + y * y
    sqh = ((sq // 1024) * 1024).astype(np.float32)
    sqm = (((sq // 32) % 32) * 32).astype(np.float32)
    sql = (sq % 32).astype(np.float32)
    xf = x.astype(np.float32)
    yf = y.astype(np.float32)
    a = (2 * org - 1).astype(np.float32)
    fi = (roles == 1).astype(np.float32)
    ma = (roles == 0).astype(np.float32)
    ones = np.ones(N, np.float32)

    # phase-1 d2 features
    lj = np.stack([xf, yf, sqh, sqm, sql, ones, ones, ones]).astype(bf)      # [8, N]
    ri_full = np.stack([-2 * xf, -2 * yf, ones, ones, ones,
                        sqh, sqm, sql]).astype(bf)                           # [8, N]

    # phase-2 key features
    cj = (np.float32(32 * BIG) - np.float32(16 * BIG) * fi).astype(np.float32)
    jloc = (np.arange(N) % 32).astype(np.float32)
    cfij = (np.float32(-16 * BIG) * a * fi).astype(np.float32)
    rj2 = np.stack([-64 * xf, -64 * yf, 32 * sqh, 32 * sqm, 32 * sql,
                    ones, ones, ones, cj, jloc, cfij]).astype(bf)            # [11, N]
    li2_full = np.stack([xf, yf, ones, ones, ones, 32 * sqh, 32 * sqm,
                         32 * sql, ones, ones, a]).astype(bf)                # [11, N]

    gj = np.stack([ma, fi, ones, a * ma, a * fi, a], axis=1).astype(bf)      # [N, 6]

    sT = states.T.astype(np.float32)                                         # [32, N]
    s_hi, s_lo = _split_hi_lo(sT)
    st3 = np.concatenate([s_hi, s_hi, s_lo], axis=0)                         # [96, N]

    w_out = np.asarray(inputs["w_out"], np.float32)
    w_in = np.asarray(inputs["w_in"], np.float32)
    W = np.concatenate([w_out[0], w_out[1], w_in[0], w_in[1]], axis=1)       # [32, 256]
    w_hi, w_lo = _split_hi_lo(W)
    w3 = np.concatenate([w_hi, w_lo, w_hi], axis=0)                          # [96, 256]

    scoffs = _dedup_scatter_offsets(x, y).reshape(128, 64)
    enat = energies.reshape(128, 64).astype(np.float32)
    potoffs_full = (y * GRID + x).astype(np.int32)                           # [N]
    segrow = (32.0 * np.arange(N // 32, dtype=np.float32)).reshape(1, 256)

    shared = dict(
        lj=lj, rj2=rj2, gj=gj, st3=st3, w3=w3,
        orgcol=org.astype(np.float32).reshape(N, 1),
        acol=a.reshape(N, 1),
        scat=np.concatenate([enat, scoffs.view(np.float32)],
                            axis=1).astype(np.float32),
        postab=pos.astype(np.int32),
        bmat=_band_matrix(),
        ident32=np.eye(32, dtype=np.float32),
        segrow=segrow,
        sw1=np.asarray(inputs["sw1"], np.float32),
        sw2=np.asarray(inputs["sw2"], np.float32),
        tw1=np.asarray(inputs["tw1"], np.float32),
        tw2=np.asarray(inputs["tw2"], np.float32),
        sb1t=np.asarray(inputs["sb1"], np.float32).T.copy(),                 # [64, 2]
        sb2t=np.asarray(inputs["sb2"], np.float32).T.copy(),                 # [32, 2]
        tb1t=np.asarray(inputs["tb1"], np.float32).T.copy(),
        tb2t=np.asarray(inputs["tb2"], np.float32).T.copy(),
    )

    per_core = []
    for c in range(NCORES):
        sl = slice(c * M, (c + 1) * M)
        cells = np.arange(c * M, (c + 1) * M)
        # i-layout [p, f*8 + b]: cell = 128*b + p (core-local)
        fields = np.stack([xf[sl], yf[sl], energies[sl],
                           roles[sl].astype(np.float32), a[sl], rand[sl],
                           org[sl].astype(np.float32), np.zeros(M, np.float32)])
        cellsc = np.zeros((128, 64), np.float32)
        for f in range(8):
            cellsc[:, f * 8:(f + 1) * 8] = fields[f].reshape(NBLK, 128).T
        per_core.append(dict(
            ric=np.ascontiguousarray(ri_full[:, sl]),
            li2c=np.ascontiguousarray(li2_full[:, sl]),
            st3c=np.ascontiguousarray(st3[:, sl]),
            statesTc=np.ascontiguousarray(sT[:, sl]),
            states_il=np.ascontiguousarray(states[sl]),
            orgrow=org[sl].astype(np.float32).reshape(1, M),
            orgcolc=org[sl].astype(np.float32).reshape(M, 1),
            cellsc=cellsc,
            potoffs=np.ascontiguousarray(
                potoffs_full[sl].reshape(NBLK, 128).T).astype(np.int32),
        ))
    return shared, per_core


# ----------------------------------------------------------------------------
# device program
# ----------------------------------------------------------------------------

def build_program():
    nc = bacc.Bacc("TRN2", target_bir_lowering=False, debug=False,
                   num_devices=NCORES)

    def din(name, shape, dtype):
        return nc.dram_tensor(name, list(shape), dtype, kind="ExternalInput").ap()

    def dout(name, shape, dtype):
        return nc.dram_tensor(name, list(shape), dtype, kind="ExternalOutput").ap()

    ins = dict(
        lj=din("lj", (8, N), BF16),
        rj2=din("rj2", (11, N), BF16),
        gj=din("gj", (N, 6), BF16),
        st3=din("st3", (96, N), BF16),
        w3=din("w3", (96, 256), BF16),
        orgcol=din("orgcol", (N, 1), F32),
        acol=din("acol", (N, 1), F32),
        scat=din("scat", (128, 128), F32),
        postab=din("postab", (N, 2), I32),
        bmat=din("bmat", (GRID, GRID), F32),
        ident32=din("ident32", (32, 32), F32),
        segrow=din("segrow", (1, 256), F32),
        sw1=din("sw1", (2, 32, H), F32),
        sw2=din("sw2", (2, H, 32), F32),
        tw1=din("tw1", (2, 33, H), F32),
        tw2=din("tw2", (2, H, 32), F32),
        sb1t=din("sb1t", (H, 2), F32),
        sb2t=din("sb2t", (32, 2), F32),
        tb1t=din("tb1t", (H, 2), F32),
        tb2t=din("tb2t", (32, 2), F32),
        ric=din("ric", (8, M), BF16),
        li2c=din("li2c", (11, M), BF16),
        st3c=din("st3c", (96, M), BF16),
        statesTc=din("statesTc", (32, M), F32),
        states_il=din("states_il", (M, 32), F32),
        orgrow=din("orgrow", (1, M), F32),
        orgcolc=din("orgcolc", (M, 1), F32),
        cellsc=din("cellsc", (128, 64), F32),
        potoffs=din("potoffs", (128, NBLK), I32),
    )
    outs = dict(
        o_states=dout("o_states", (NBLK, 128, 32), F32),
        o_e=dout("o_e", (128, NBLK), F32),
        o_roles=dout("o_roles", (128, NBLK), I32),
        o_pos=dout("o_pos", (NBLK, 128, 2), I32),
        o_org=dout("o_org", (128, NBLK), I32),
    )

    egrid_d = nc.dram_tensor("egrid_scratch", [GRID * GRID + 1, 1], F32,
                             kind="Internal").ap()
    field_d = nc.dram_tensor("field_scratch", [GRID * GRID + 1, 1], F32,
                             kind="Internal").ap()
    potrow_d = nc.dram_tensor("potrow_scratch", [1, M], F32,
                              kind="Internal").ap()

    with tile.TileContext(nc) as tc:
        _build(nc, tc, ins, outs, egrid_d, field_d, potrow_d)

    nc.compile()
    return nc


def _build(nc, tc, ins, outs, egrid_d, field_d, potrow_d):
    import contextlib
    ctx = contextlib.ExitStack()
    with ctx:
        _build_inner(nc, tc, ctx, ins, outs, egrid_d, field_d, potrow_d)


def _build_inner(nc, tc, ctx, ins, outs, egrid_d, field_d, potrow_d):
    # ---- pools ----
    # PSUM: 8 banks total. pd2(2) + pkey(2, shared tag with misc) + pagg(4).
    pd2 = ctx.enter_context(tc.tile_pool(name="pd2", bufs=2, space="PSUM"))
    pkey = ctx.enter_context(tc.tile_pool(name="pkey", bufs=2, space="PSUM"))
    pagg = ctx.enter_context(tc.tile_pool(name="pagg", bufs=4, space="PSUM"))
    sb = ctx.enter_context(tc.tile_pool(name="sb", bufs=2))        # streaming
    sb3 = ctx.enter_context(tc.tile_pool(name="sb3", bufs=3))      # S tiles
    pmlp = ctx.enter_context(tc.tile_pool(name="pmlp", bufs=1))
    pfp = ctx.enter_context(tc.tile_pool(name="pfp", bufs=8))
    pblur = ctx.enter_context(tc.tile_pool(name="pblur", bufs=6))
    pers = ctx.enter_context(tc.tile_pool(name="pers", bufs=1))    # persistent

    _uid = [0]

    def pt(pool, shape, dtype, tag):
        _uid[0] += 1
        return pool.tile(list(shape), dtype, tag=tag,
                         name=f"{tag}_{_uid[0]}")

    # ---- persistent SBUF tensors ----
    w3t = pt(pers, (96, 256), BF16, "w3t")
    rit = pt(pers, (8, M), BF16, "rit")
    li2t = pt(pers, (11, M), BF16, "li2t")
    st3ct = pt(pers, (96, M), BF16, "st3ct")
    phi = pt(pers, (128, NCH * PC2), BF16, "phi")      # P hi, chunk-major
    plo = pt(pers, (128, NCH * PC2), BF16, "plo")
    pselfown = pt(pers, (128, NBLK * 128), F32, "pselfown")
    aggacc = pt(pers, (128, NBLK * PC2), F32, "aggacc")
    segkeys = pt(pers, (128, NBLK * 256), F32, "segkeys")
    cellsct = pt(pers, (128, 64), F32, "cellsct")
    grow = pt(pers, (1, PC2), F32, "grow")
    grep = pt(pers, (128, PC2), F32, "grep")
    hugetile = pt(pers, (128, 256), F32, "hugetile")
    segrep = pt(pers, (128, 256), F32, "segrep")
    ones128b = pt(pers, (128, 1), BF16, "ones128b")
    ones1f = pt(pers, (1, 128), F32, "ones1f")
    bias255 = pt(pers, (128, 1), F32, "bias255")
    nc.vector.memset(bias255[:], 25.5)

    nc.sync.dma_start(w3t[:], ins["w3"])
    nc.sync.dma_start(rit[:], ins["ric"])
    nc.sync.dma_start(li2t[:], ins["li2c"])
    nc.sync.dma_start(st3ct[:], ins["st3c"])
    nc.sync.dma_start(cellsct[:], ins["cellsc"])
    nc.vector.memset(hugetile[:], HUGE)
    nc.vector.memset(ones128b[:], 1.0)
    nc.vector.memset(ones1f[:], 1.0)
    # segrep = broadcast of segrow over partitions (DMA broadcast read)
    nc.sync.dma_start(segrep[:], ins["segrow"].to_broadcast([128, 256]))

    # i-layout convenience views of cellsc
    xf_il = cellsct[:, 0:8]
    yf_il = cellsct[:, 8:16]
    e_il = cellsct[:, 16:24]
    roles_il = cellsct[:, 24:32]
    a_il = cellsct[:, 32:40]
    rand_il = cellsct[:, 40:48]
    org_il = cellsct[:, 48:56]

    # =======================================================================
    # force field: zero grid -> scatter -> blur -> gather
    # =======================================================================
    scratch512 = pt(pers, (128, 512), F32, "scratch512")
    ztile = scratch512
    nc.vector.memset(ztile[:], 0.0)
    eg_flat = egrid_d[0:GRID * GRID, :].rearrange("(p f) o -> p (f o)", p=128)
    zdma = nc.gpsimd.dma_start(eg_flat, ztile[:])
    # slot GRID*GRID (the dedup trash slot) is written by losers and never
    # read, so it needs no zero-init.

    scat_t = pt(pers, (128, 128), F32, "scat_t")
    nc.sync.dma_start(scat_t[:], ins["scat"])
    # Indirect (qPoolDynamic) DMAs may carry at most ONE sync wait in walrus
    # codegen. Funnel every dependency through one Pool compute gate: the
    # gate waits on all of them, the indirect DMA then waits only on Pool.
    g1 = nc.vector.tensor_copy(scat_t[0:1, 0:1], scat_t[0:1, 0:1])
    bass._add_dep_helper(g1.ins, zdma.ins, True, "funnel scatter deps")
    nc.gpsimd.indirect_dma_start(
        out=egrid_d,
        out_offset=IndirectOffsetOnAxis(
            ap=scat_t[:, 64:128].bitcast(I32), axis=0),
        in_=scat_t[:, 0:64],
        in_offset=None,
    )

    bt = [pt(pblur, (128, GRID), F32, "blur") for r in range(2)]
    egt = [pt(pblur, (128, GRID), F32, "blur") for r in range(2)]
    t1s = [pt(pblur, (128, GRID), F32, "blur") for r in range(2)]

    for r in range(2):
        nc.sync.dma_start(bt[r][:], ins["bmat"][128 * r:128 * (r + 1), :])
        nc.sync.dma_start(
            egt[r][:],
            egrid_d[128 * GRID * r:128 * GRID * (r + 1), :]
            .rearrange("(p f) o -> p (f o)", p=128))
    # T1'[c, u] = sum_r egrid[r, c] * B[r, u]
    for cb in range(2):
        ps = pt(pkey, (128, GRID), F32, "shared")
        for r in range(2):
            nc.tensor.matmul(ps[:], lhsT=egt[r][:, 128 * cb:128 * (cb + 1)],
                             rhs=bt[r][:], start=(r == 0), stop=(r == 1))
        nc.scalar.copy(t1s[cb][:], ps[:])
    # field[u, v] = sum_c T1'[c, u] * B[c, v]
    fboth = pt(pblur, (128, 2 * GRID), F32, "blur")
    for ub in range(2):
        ps = pt(pkey, (128, GRID), F32, "shared")
        for cc in range(2):
            nc.tensor.matmul(ps[:], lhsT=t1s[cc][:, 128 * ub:128 * (ub + 1)],
                             rhs=bt[cc][:], start=(cc == 0), stop=(cc == 1))
        nc.scalar.copy(fboth[:, GRID * ub:GRID * (ub + 1)], ps[:])
    fdma = nc.sync.dma_start(
        field_d[0:2 * 128 * GRID, :]
        .rearrange("(u p v) o -> p u (v o)", u=2, p=128),
        fboth[:].rearrange("p (u v) -> p u v", u=2))

    potoffs_t = pt(pers, (128, NBLK), I32, "potoffs_t")
    nc.sync.dma_start(potoffs_t[:], ins["potoffs"])
    # same single-wait funnel as the scatter above
    g2 = nc.vector.tensor_copy(potoffs_t[0:1, 0:1], potoffs_t[0:1, 0:1])
    bass._add_dep_helper(g2.ins, fdma.ins, True, "funnel gather deps")
    pot_il = pt(pers, (128, NBLK), F32, "pot_il")
    nc.gpsimd.indirect_dma_start(
        out=pot_il[:], out_offset=None, in_=field_d,
        in_offset=IndirectOffsetOnAxis(ap=potoffs_t[:], axis=0))
    # bounce potential to a [1, M] row for the transform MLP input
    nc.sync.dma_start(
        potrow_d[0, :].rearrange("(b p) -> p b", p=128), pot_il[:])

    # =======================================================================
    # P build (all 64 chunks) + G row + own psel
    # =======================================================================
    # zero the G columns of plo once (strided views)
    nc.vector.memset(
        plo[:].rearrange("p (c k) -> p c k", k=PC2)[:, :, 128:131], 0.0)
    nc.vector.memset(
        plo[:].rearrange("p (c k) -> p c k", k=PC2)[:, :, 259:262], 0.0)

    gacc = pt(pers, (1, PC2), F32, "gacc")
    nc.vector.memset(gacc[:], 0.0)

    for ch in range(NCH):
        base = ch * PC2
        st3c_t = pt(sb, (96, 128), BF16, "st3c_t")
        nc.sync.dma_start(st3c_t[:], ins["st3"][:, 128 * ch:128 * (ch + 1)])
        psP = pt(pkey, (128, 256), F32, "shared")
        nc.tensor.matmul(psP[:], lhsT=st3c_t[:],
                         rhs=w3t[:], start=True, stop=True)
        pself = pt(sb, (128, 256), F32, "pself")
        nc.scalar.activation(pself[:], psP[:], ACTF.Tanh)
        orgch = pt(sb, (128, 1), F32, "orgch")
        nc.sync.dma_start(orgch[:], ins["orgcol"][128 * ch:128 * (ch + 1), :])
        ach = pt(sb, (128, 1), F32, "ach")
        nc.sync.dma_start(ach[:], ins["acol"][128 * ch:128 * (ch + 1), :])

        v4 = pself[:].rearrange("p (b k) -> p b k", k=64)
        p0 = v4[:, 0::2, :]    # cols of p0o | p0i
        p1 = v4[:, 1::2, :]    # cols of p1o | p1i
        dsel = pt(sb, (128, 128), F32, "dsel")
        dv = dsel[:].rearrange("p (b k) -> p b k", k=64)
        nc.vector.tensor_tensor(dv, p1, p0, op=OP.subtract)
        pself_sel = pt(sb, (128, 128), F32, "pself_sel")
        pv = pself_sel[:].rearrange("p (b k) -> p b k", k=64)
        nc.vector.scalar_tensor_tensor(pv, in0=dv, scalar=orgch[:],
                                       in1=p0, op0=OP.mult, op1=OP.add)
        # splits into phi/plo + a-scaled copies
        hi_sl = phi[:, base:base + 128]
        nc.vector.tensor_copy(hi_sl, pself_sel[:])
        nc.vector.tensor_tensor(plo[:, base:base + 128], pself_sel[:], hi_sl,
                                op=OP.subtract)
        nc.vector.tensor_scalar_mul(phi[:, base + PC:base + PC + 128],
                                    hi_sl, ach[:])
        nc.vector.tensor_scalar_mul(plo[:, base + PC:base + PC + 128],
                                    plo[:, base:base + 128], ach[:])
        nc.sync.dma_start(phi[:, base + 128:base + 131],
                          ins["gj"][128 * ch:128 * (ch + 1), 0:3])
        nc.sync.dma_start(phi[:, base + 259:base + 262],
                          ins["gj"][128 * ch:128 * (ch + 1), 3:6])

    # G row: column sums of P via ones-vector matmuls, windowed into SBUF
    for w in range(8):
        psG = pt(pkey, (1, PC2), F32, "shared")
        for k in range(8):
            ch = w * 8 + k
            nc.tensor.matmul(psG[:], lhsT=ones128b[:],
                             rhs=phi[:, ch * PC2:(ch + 1) * PC2],
                             start=(k == 0), stop=False)
            nc.tensor.matmul(psG[:], lhsT=ones128b[:],
                             rhs=plo[:, ch * PC2:(ch + 1) * PC2],
                             start=False, stop=(k == 7))
        nc.vector.tensor_tensor(gacc[:], psG[:], gacc[:], op=OP.add)
    # broadcast G row to 128 partitions via fp32 matmul
    psB = pt(pkey, (128, PC2), F32, "shared")
    nc.tensor.matmul(psB[:], lhsT=ones1f[:], rhs=gacc[:], start=True, stop=True)
    nc.scalar.copy(grep[:], psB[:])

    # own psel (for P_i correction and the mean-score dots)
    for t in range(NBLK):
        psP = pt(pkey, (128, 256), F32, "shared")
        nc.tensor.matmul(psP[:], lhsT=st3ct[:, 128 * t:128 * (t + 1)],
                         rhs=w3t[:], start=True, stop=True)
        pself = pt(sb, (128, 256), F32, "pself")
        nc.scalar.activation(pself[:], psP[:], ACTF.Tanh)
        orgch = pt(sb, (128, 1), F32, "orgch")
        nc.sync.dma_start(orgch[:], ins["orgcolc"][128 * t:128 * (t + 1), :])
        v4 = pself[:].rearrange("p (b k) -> p b k", k=64)
        p0 = v4[:, 0::2, :]
        p1 = v4[:, 1::2, :]
        dsel = pt(sb, (128, 128), F32, "dsel")
        dv = dsel[:].rearrange("p (b k) -> p b k", k=64)
        nc.vector.tensor_tensor(dv, p1, p0, op=OP.subtract)
        pv = pselfown[:, 128 * t:128 * (t + 1)].rearrange(
            "p (b k) -> p b k", k=64)
        nc.vector.scalar_tensor_tensor(pv, in0=dv, scalar=orgch[:],
                                       in1=p0, op0=OP.mult, op1=OP.add)

    # =======================================================================
    # phase 1: masks + aggregation.  2 passes x 4 blocks.
    # =======================================================================
    nc.vector.memset(aggacc[:], 0.0)
    for q in range(2):
        for w in range(8):                    # 8 windows of 8 chunks
            aps = [pt(pagg, (128, PC2), F32, "agg") for b in range(4)]
            for k in range(8):
                ch = w * 8 + k
                ljc = pt(sb, (8, 128), BF16, "ljc")
                nc.sync.dma_start(ljc[:], ins["lj"][:, 128 * ch:128 * (ch + 1)])
                psD = pt(pd2, (128, 512), F32, "d2")
                nc.tensor.matmul(psD[:], lhsT=ljc[:],
                                 rhs=rit[:, 512 * q:512 * (q + 1)],
                                 start=True, stop=True)
                stile = pt(sb3, (128, 512), BF16, "stile")
                nc.scalar.activation(stile[:], psD[:], ACTF.Sign,
                                     bias=bias255[:], scale=-1.0)
                for b in range(4):
                    blk = q * 4 + b
                    nc.tensor.matmul(
                        aps[b][:], lhsT=stile[:, 128 * b:128 * (b + 1)],
                        rhs=phi[:, ch * PC2:(ch + 1) * PC2],
                        start=(k == 0), stop=False)
                    nc.tensor.matmul(
                        aps[b][:], lhsT=stile[:, 128 * b:128 * (b + 1)],
                        rhs=plo[:, ch * PC2:(ch + 1) * PC2],
                        start=False, stop=(k == 7))
            for b in range(4):
                blk = q * 4 + b
                acc = aggacc[:, blk * PC2:(blk + 1) * PC2]
                nc.vector.tensor_tensor(acc, aps[b][:], acc, op=OP.add)

    # =======================================================================
    # phase 2: nearest same-org FI via exact integer keys
    # =======================================================================
    for blk in range(NBLK):
        segs = segkeys[:, blk * 256:(blk + 1) * 256]
        for k in range(16):
            rj2c = pt(sb, (11, 512), BF16, "rj2c")
            nc.sync.dma_start(rj2c[:], ins["rj2"][:, 512 * k:512 * (k + 1)])
            psK = pt(pkey, (128, 512), F32, "shared")
            nc.tensor.matmul(psK[:], lhsT=li2t[:, 128 * blk:128 * (blk + 1)],
                             rhs=rj2c[:],
                             start=True, stop=True)
            nc.vector.tensor_reduce(
                segs[:, 16 * k:16 * (k + 1)].rearrange("p (f o) -> p f o", o=1),
                psK[:].rearrange("p (s e) -> p s e", e=32),
                axis=AX.X, op=OP.min)

    # per-block post: m, elig, first index
    nearest_il = pt(pers, (128, NBLK), F32, "nearest_il")
    hfc_il = pt(pers, (128, NBLK), F32, "hfc_il")
    for blk in range(NBLK):
        segs = segkeys[:, blk * 256:(blk + 1) * 256]
        m32k = pt(sb, (128, 1), F32, "m32k")
        nc.vector.tensor_reduce(m32k[:], segs, axis=AX.X, op=OP.min)
        # m32 = 32*floor(m32k/32) without mod/floor ALU ops: int-convert
        # (rounding-mode agnostic) then compare-fixup the possible overshoot.
        q1f = pt(sb, (128, 1), F32, "q1f")
        nc.vector.tensor_scalar(q1f[:], m32k[:], 1.0 / 32.0, None, op0=OP.mult)
        q1i = pt(sb, (128, 1), I32, "q1i")
        nc.vector.tensor_copy(q1i[:], q1f[:])
        nc.vector.tensor_copy(q1f[:], q1i[:])
        q32 = pt(sb, (128, 1), F32, "q32")
        nc.vector.tensor_scalar(q32[:], q1f[:], 32.0, None, op0=OP.mult)
        fix = pt(sb, (128, 1), F32, "fix")
        nc.vector.tensor_tensor(fix[:], q32[:], m32k[:], op=OP.is_gt)
        m32 = pt(sb, (128, 1), F32, "m32")
        nc.vector.tensor_scalar(fix[:], fix[:], -32.0, None, op0=OP.mult)
        nc.vector.tensor_tensor(m32[:], q32[:], fix[:], op=OP.add)
        t1k = pt(sb, (128, 256), F32, "t1k")
        nc.vector.tensor_scalar(t1k[:], segs, m32[:], None, op0=OP.subtract)
        elig = pt(sb, (128, 256), F32, "elig")
        nc.vector.tensor_scalar(elig[:], t1k[:], 31.0, None, op0=OP.is_le)
        m1 = pt(sb, (128, 256), F32, "m1")
        nc.vector.scalar_tensor_tensor(m1[:], in0=elig[:], scalar=-HUGE,
                                       in1=hugetile[:], op0=OP.mult, op1=OP.add)
        t2 = pt(sb, (128, 256), F32, "t2")
        nc.vector.tensor_tensor(t2[:], t1k[:], segrep[:], op=OP.add)
        key3 = pt(sb, (128, 256), F32, "key3")
        nc.vector.tensor_tensor(key3[:], t2[:], m1[:], op=OP.add)
        nc.vector.tensor_reduce(nearest_il[:, blk:blk + 1], key3[:],
                                axis=AX.X, op=OP.min)
        # rows with no candidate have key3 >= HUGE: clamp before the gather
        nc.vector.tensor_scalar(nearest_il[:, blk:blk + 1],
                                nearest_il[:, blk:blk + 1], float(N - 1),
                                None, op0=OP.min)
        nc.vector.tensor_scalar(hfc_il[:, blk:blk + 1], m32[:],
                                float(32 * BIG), None, op0=OP.is_lt)

    # gather fpos
    nearest_i32 = pt(pers, (128, NBLK), I32, "nearest_i32")
    nc.vector.tensor_copy(nearest_i32[:], nearest_il[:])
    fx_il = pt(pers, (128, NBLK), F32, "fx_il")
    fy_il = pt(pers, (128, NBLK), F32, "fy_il")
    for blk in range(NBLK):
        fp = pt(pfp, (128, 2), I32, "fp")
        nc.gpsimd.indirect_dma_start(
            out=fp[:], out_offset=None, in_=ins["postab"],
            in_offset=IndirectOffsetOnAxis(
                ap=nearest_i32[:, blk:blk + 1], axis=0))
        nc.vector.tensor_copy(fx_il[:, blk:blk + 1], fp[:, 0:1])
        nc.vector.tensor_copy(fy_il[:, blk:blk + 1], fp[:, 1:2])

    # =======================================================================
    # aggregation post: sameagg cols, counts, dots
    # =======================================================================
    samemass_il = pt(pers, (128, NBLK), F32, "samemass_il")
    samefi_il = pt(pers, (128, NBLK), F32, "samefi_il")
    cnt_il = pt(pers, (128, NBLK), F32, "cnt_il")
    otherfi_il = pt(pers, (128, NBLK), F32, "otherfi_il")
    dot_o_il = pt(pers, (128, NBLK), F32, "dot_o_il")
    dot_i_il = pt(pers, (128, NBLK), F32, "dot_i_il")
    ismass_il = pt(pers, (128, NBLK), F32, "ismass_il")
    nc.vector.tensor_scalar(ismass_il[:], roles_il, -1.0, 1.0,
                            op0=OP.mult, op1=OP.add)
    na_il = pt(pers, (128, NBLK), F32, "na_il")
    nc.vector.tensor_scalar(na_il[:], a_il, -1.0, None, op0=OP.mult)
    trash = scratch512

    for blk in range(NBLK):
        acc = aggacc[:, blk * PC2:(blk + 1) * PC2]
        a_col = a_il[:, blk:blk + 1]
        U = pt(sb, (128, PC2), F32, "U")
        nc.vector.tensor_tensor(U[:], acc, grep[:], op=OP.add)
        t1 = pt(sb, (128, PC), F32, "t1")
        nc.vector.scalar_tensor_tensor(t1[:], in0=U[:, PC:PC2], scalar=a_col,
                                       in1=U[:, 0:PC], op0=OP.mult, op1=OP.add)
        sagg = pt(sb, (128, 128), F32, "sagg")
        nc.vector.scalar_tensor_tensor(
            sagg[:], in0=t1[:, 0:128], scalar=0.25,
            in1=pselfown[:, 128 * blk:128 * (blk + 1)],
            op0=OP.mult, op1=OP.subtract)
        # counts: same_mass, same_fi (subtract own indicator), cnt (minus 1)
        nc.vector.scalar_tensor_tensor(
            samemass_il[:, blk:blk + 1], in0=t1[:, 128:129], scalar=0.25,
            in1=ismass_il[:, blk:blk + 1], op0=OP.mult, op1=OP.subtract)
        nc.vector.scalar_tensor_tensor(
            samefi_il[:, blk:blk + 1], in0=t1[:, 129:130], scalar=0.25,
            in1=roles_il[:, blk:blk + 1], op0=OP.mult, op1=OP.subtract)
        nc.vector.tensor_scalar(cnt_il[:, blk:blk + 1], t1[:, 130:131],
                                0.25, -1.0, op0=OP.mult, op1=OP.add)
        # other_fi = 0.25 * (U_fi - a * U_afi)
        tof = pt(sb, (128, 1), F32, "tof")
        nc.vector.scalar_tensor_tensor(
            tof[:], in0=U[:, PC + 129:PC + 130], scalar=na_il[:, blk:blk + 1],
            in1=U[:, 129:130], op0=OP.mult, op1=OP.add)
        nc.vector.tensor_scalar(otherfi_il[:, blk:blk + 1], tof[:], 0.25,
                                None, op0=OP.mult)
        # dots
        nc.vector.scalar_tensor_tensor(
            trash[:, 0:64], in0=sagg[:, 0:64], scalar=1.0,
            in1=pselfown[:, 128 * blk:128 * blk + 64],
            op0=OP.bypass, op1=OP.mult,
            accum_out=dot_o_il[:, blk:blk + 1])
        nc.vector.scalar_tensor_tensor(
            trash[:, 64:128], in0=sagg[:, 64:128], scalar=1.0,
            in1=pselfown[:, 128 * blk + 64:128 * blk + 128],
            op0=OP.bypass, op1=OP.mult,
            accum_out=dot_i_il[:, blk:blk + 1])

    # =======================================================================
    # MLPs in transposed layout
    # =======================================================================
    sw1t = [pt(pers, (32, H), F32, f"sw1t{o}") for o in range(2)]
    sw2t = [pt(pers, (H, 32), F32, f"sw2t{o}") for o in range(2)]
    tw1t = [pt(pers, (33, H), F32, f"tw1t{o}") for o in range(2)]
    tw2t = [pt(pers, (H, 32), F32, f"tw2t{o}") for o in range(2)]
    sb1tt = pt(pers, (H, 2), F32, "sb1tt")
    sb2tt = pt(pers, (32, 2), F32, "sb2tt")
    tb1tt = pt(pers, (H, 2), F32, "tb1tt")
    tb2tt = pt(pers, (32, 2), F32, "tb2tt")
    for o in range(2):
        nc.sync.dma_start(sw1t[o][:], ins["sw1"][o])
        nc.sync.dma_start(sw2t[o][:], ins["sw2"][o])
        nc.sync.dma_start(tw1t[o][:], ins["tw1"][o])
        nc.sync.dma_start(tw2t[o][:], ins["tw2"][o])
    nc.sync.dma_start(sb1tt[:], ins["sb1t"])
    nc.sync.dma_start(sb2tt[:], ins["sb2t"])
    nc.sync.dma_start(tb1tt[:], ins["tb1t"])
    nc.sync.dma_start(tb2tt[:], ins["tb2t"])

    statesTt = pt(pers, (32, M), F32, "statesTt")
    nc.sync.dma_start(statesTt[:], ins["statesTc"])
    orgrept = pt(pers, (32, M), F32, "orgrept")
    nc.sync.dma_start(orgrept[:], ins["orgrow"].to_broadcast([32, M]))
    ident32t = pt(pers, (32, 32), F32, "ident32t")
    nc.sync.dma_start(ident32t[:], ins["ident32"])
    states_ilt = pt(pers, (128, NBLK * 32), F32, "states_ilt")
    for blk in range(NBLK):
        nc.sync.dma_start(states_ilt[:, 32 * blk:32 * (blk + 1)],
                          ins["states_il"][128 * blk:128 * (blk + 1), :])

    def mlp_T(v_t, kdim, w1, b1t, w2, b2t, out_t):
        """out_t [32, M] = select_org( tanh(v @ W1 + b1) @ W2 + b2 )."""
        tr0 = pt(pmlp, (32, M), F32, "tr0")
        tr1 = pt(pmlp, (32, M), F32, "tr1")
        tr = [tr0[:], tr1[:]]
        for o in range(2):
            for h2 in range(2):
                ht = pt(pmlp, (H, 512), F32, "ht")
                psH = pt(pkey, (H, 512), F32, "shared")
                nc.tensor.matmul(psH[:], lhsT=w1[o][:],
                                 rhs=v_t[:, 512 * h2:512 * (h2 + 1)],
                                 start=True, stop=True)
                nc.scalar.activation(ht[:], psH[:],
                                     ACTF.Tanh, bias=b1t[:, o:o + 1])
                psT = pt(pkey, (32, 512), F32, "shared")
                nc.tensor.matmul(psT[:], lhsT=w2[o][:],
                                 rhs=ht[:],
                                 start=True, stop=True)
                nc.scalar.activation(tr[o][:, 512 * h2:512 * (h2 + 1)],
                                     psT[:], ACTF.Identity,
                                     bias=b2t[:, o:o + 1])
        dT = pt(pmlp, (32, M), F32, "dT")
        nc.vector.tensor_tensor(dT[:], tr[1], tr[0], op=OP.subtract)
        nc.vector.tensor_tensor(dT[:], dT[:], orgrept[:], op=OP.mult)
        nc.vector.tensor_tensor(out_t[:], dT[:], tr[0], op=OP.add)

    spT = pt(pers, (32, M), F32, "spT")
    mlp_T(statesTt, 32, sw1t, sb1tt, sw2t, sb2tt, spT)

    vin = pt(pers, (33, M), F32, "vin")
    nc.vector.scalar_tensor_tensor(vin[0:32, :], in0=spT[:], scalar=0.1,
                                   in1=statesTt[:], op0=OP.mult, op1=OP.add)
    nc.sync.dma_start(vin[32:33, :], potrow_d)

    trT = pt(pers, (32, M), F32, "trT")
    mlp_T(vin, 33, tw1t, tb1tt, tw2t, tb2tt, trT)

    # =======================================================================
    # final elementwise stage (i-layout [128, 8])
    # =======================================================================
    def ts(out, in0, s1, s2=None, op0=OP.mult, op1=None):
        if s2 is None:
            nc.vector.tensor_scalar(out, in0, s1, None, op0=op0)
        else:
            nc.vector.tensor_scalar(out, in0, s1, s2, op0=op0, op1=op1)

    def tt(out, i0, i1, op):
        nc.vector.tensor_tensor(out, i0, i1, op=op)

    def stt(out, in0, scalar, in1, op0, op1):
        nc.vector.scalar_tensor_tensor(out, in0=in0, scalar=scalar, in1=in1,
                                       op0=op0, op1=op1)

    def til(tag):
        return pt(sb, (128, NBLK), F32, tag)

    cnt_safe = til("cnt_safe"); ts(cnt_safe[:], cnt_il[:], 1.0, op0=OP.max)
    hasnb = til("hasnb"); ts(hasnb[:], cnt_il[:], 0.0, op0=OP.is_gt)
    rcnt = til("rcnt"); nc.vector.reciprocal(rcnt[:], cnt_safe[:])
    om = til("om"); tt(om[:], dot_o_il[:], rcnt[:], OP.mult)
    im = til("im"); tt(im[:], dot_i_il[:], rcnt[:], OP.mult)
    netinf = til("netinf")
    tt(netinf[:], om[:], im[:], OP.subtract)
    tt(netinf[:], netinf[:], hasnb[:], OP.mult)
    score = til("score"); ts(score[:], netinf[:], 1.0, 0.5, op0=OP.mult, op1=OP.add)

    # energies
    fi_e = til("fi_e")
    stt(fi_e[:], samemass_il[:], 0.02, e_il, OP.mult, OP.add)
    stt(fi_e[:], otherfi_il[:], -0.03, fi_e[:], OP.mult, OP.add)
    me = til("me")
    ts(me[:], e_il, 0.995)
    pp = til("pp"); ts(pp[:], pot_il[:], 0.0, op0=OP.max)
    stt(me[:], pp[:], 0.05, me[:], OP.mult, OP.add)
    sfgt = til("sfgt"); ts(sfgt[:], samefi_il[:], 0.0, op0=OP.is_gt)
    stt(me[:], sfgt[:], 0.02, me[:], OP.mult, OP.add)
    ofgt = til("ofgt"); ts(ofgt[:], otherfi_il[:], 0.0, op0=OP.is_gt)
    stt(me[:], ofgt[:], -0.01, me[:], OP.mult, OP.add)
    newe = til("newe")
    tt(newe[:], fi_e[:], me[:], OP.subtract)
    tt(newe[:], newe[:], roles_il, OP.mult)       # roles in {0,1}: is_fi
    tt(newe[:], newe[:], me[:], OP.add)
    sp = til("sp"); ts(sp[:], score[:], 0.0, op0=OP.max)
    stt(newe[:], sp[:], 0.02, newe[:], OP.mult, OP.add)
    ts(newe[:], newe[:], 0.0, op0=OP.max)
    ts(newe[:], newe[:], 1.0, op0=OP.min)

    # roles
    c1 = til("c1"); ts(c1[:], newe[:], 0.5, op0=OP.is_gt)
    c2 = til("c2"); ts(c2[:], samemass_il[:], 2.0, op0=OP.is_ge)
    c3a = til("c3a"); ts(c3a[:], samefi_il[:], 0.0, op0=OP.is_equal)
    c3b = til("c3b"); ts(c3b[:], score[:], 0.3, op0=OP.is_gt)
    c3 = til("c3"); tt(c3[:], c3a[:], c3b[:], OP.max)
    c4 = til("c4"); ts(c4[:], otherfi_il[:], 0.0, op0=OP.is_equal)
    canfi = til("canfi")
    tt(canfi[:], c1[:], c2[:], OP.mult)
    tt(canfi[:], canfi[:], c3[:], OP.mult)
    tt(canfi[:], canfi[:], c4[:], OP.mult)
    l1 = til("l1"); ts(l1[:], samemass_il[:], 1.0, op0=OP.is_lt)
    l2 = til("l2"); ts(l2[:], newe[:], 0.2, op0=OP.is_lt)
    l3 = til("l3"); tt(l3[:], otherfi_il[:], samefi_il[:], OP.is_ge)
    tt(l3[:], l3[:], ofgt[:], OP.mult)
    loses = til("loses")
    tt(loses[:], l1[:], l2[:], OP.max)
    tt(loses[:], loses[:], l3[:], OP.max)
    keeps = til("keeps"); ts(keeps[:], loses[:], -1.0, 1.0, op0=OP.mult, op1=OP.add)
    newr = til("newr")
    tt(newr[:], canfi[:], keeps[:], OP.subtract)
    tt(newr[:], newr[:], ismass_il[:], OP.mult)   # ismass = roles==0
    tt(newr[:], newr[:], keeps[:], OP.add)        # r0 ? canfi : keeps

    # movement
    nr0 = til("nr0"); ts(nr0[:], newr[:], 0.0, op0=OP.is_equal)
    move = til("move"); tt(move[:], nr0[:], hfc_il[:], OP.mult)
    dxs = til("dxs"); dys = til("dys")
    dtmp = til("dtmp"); g1 = til("g1"); g2 = til("g2")
    tt(dtmp[:], fx_il[:], xf_il, OP.subtract)
    ts(g1[:], dtmp[:], 0.0, op0=OP.is_gt)
    ts(g2[:], dtmp[:], 0.0, op0=OP.is_lt)
    tt(dxs[:], g1[:], g2[:], OP.subtract)
    tt(dtmp[:], fy_il[:], yf_il, OP.subtract)
    ts(g1[:], dtmp[:], 0.0, op0=OP.is_gt)
    ts(g2[:], dtmp[:], 0.0, op0=OP.is_lt)
    tt(dys[:], g1[:], g2[:], OP.subtract)
    nx = til("nx"); ny = til("ny")
    tt(dxs[:], dxs[:], move[:], OP.mult)
    tt(dys[:], dys[:], move[:], OP.mult)
    tt(nx[:], xf_il, dxs[:], OP.add)
    tt(ny[:], yf_il, dys[:], OP.add)
    for t_ in (nx, ny):
        ts(t_[:], t_[:], 0.0, op0=OP.max)
        ts(t_[:], t_[:], float(GRID - 1), op0=OP.min)

    # defection
    d3 = til("d3"); ts(d3[:], otherfi_il[:], 2.0, op0=OP.is_ge)
    d4 = til("d4"); ts(d4[:], rand_il, 0.1, op0=OP.is_lt)
    defect = til("defect")
    tt(defect[:], ismass_il[:], c3a[:], OP.mult)
    tt(defect[:], defect[:], d3[:], OP.mult)
    tt(defect[:], defect[:], d4[:], OP.mult)
    flip = til("flip"); ts(flip[:], org_il, -2.0, 1.0, op0=OP.mult, op1=OP.add)
    neworg = til("neworg")
    tt(flip[:], flip[:], defect[:], OP.mult)
    tt(neworg[:], org_il, flip[:], OP.add)

    # =======================================================================
    # outputs
    # =======================================================================
    nc.sync.dma_start(outs["o_e"], newe[:])
    newr_i = pt(sb, (128, NBLK), I32, "newr_i")
    nc.vector.tensor_copy(newr_i[:], newr[:])
    nc.sync.dma_start(outs["o_roles"], newr_i[:])
    neworg_i = pt(sb, (128, NBLK), I32, "neworg_i")
    nc.vector.tensor_copy(neworg_i[:], neworg[:])
    nc.sync.dma_start(outs["o_org"], neworg_i[:])

    for blk in range(NBLK):
        posf = pt(sb, (128, 2), F32, "posf")
        nc.vector.tensor_copy(posf[:, 0:1], nx[:, blk:blk + 1])
        nc.vector.tensor_copy(posf[:, 1:2], ny[:, blk:blk + 1])
        posi = pt(sb, (128, 2), I32, "posi")
        nc.vector.tensor_copy(posi[:], posf[:])
        nc.sync.dma_start(outs["o_pos"][blk], posi[:])

    # new_states per block: transpose trT slice -> [128, 32], combine, DMA out
    for blk in range(NBLK):
        psX = pt(pkey, (128, 32), F32, "shared")
        nc.tensor.transpose(psX[:], trT[:, 128 * blk:128 * (blk + 1)],
                            ident32t[:])
        s_blk = states_ilt[:, 32 * blk:32 * (blk + 1)]
        wv = pt(sb, (128, 32), F32, "wv")
        nc.vector.scalar_tensor_tensor(wv[:], in0=s_blk, scalar=0.3,
                                       in1=psX[:], op0=OP.mult, op1=OP.add)
        nc.vector.tensor_tensor(wv[:], wv[:], s_blk, op=OP.subtract)
        nst = pt(sb, (128, 32), F32, "nst")
        nc.vector.scalar_tensor_tensor(nst[:], in0=wv[:],
                                       scalar=hasnb[:, blk:blk + 1],
                                       in1=s_blk, op0=OP.mult, op1=OP.add)
        nc.sync.dma_start(outs["o_states"][blk], nst[:])


# ----------------------------------------------------------------------------
# entry point
# ----------------------------------------------------------------------------

_CACHE = {}
TRACE = False
LAST_RESULT = None


def _get_program():
    if "nc" not in _CACHE:
        _CACHE["nc"] = build_program()
    return _CACHE["nc"]


def kernel(**inputs):
    global LAST_RESULT
    from concourse.bass_utils import run_bass_kernel_spmd

    nc = _get_program()
    shared, per_core = host_prep(inputs)
    in_maps = []
    for c in range(NCORES):
        m = dict(shared)
        m.update(per_core[c])
        in_maps.append(m)
    res = run_bass_kernel_spmd(nc, in_maps, core_ids=list(range(NCORES)),
                               trace=TRACE)
    LAST_RESULT = res

    new_states = np.concatenate(
        [res.results[c]["o_states"].reshape(M, 32) for c in range(NCORES)])
    new_e = np.concatenate(
        [res.results[c]["o_e"].T.reshape(M) for c in range(NCORES)])
    new_roles = np.concatenate(
        [res.results[c]["o_roles"].T.reshape(M) for c in range(NCORES)])
    new_pos = np.concatenate(
        [res.results[c]["o_pos"].reshape(M, 2) for c in range(NCORES)])
    new_org = np.concatenate(
        [res.results[c]["o_org"].T.reshape(M) for c in range(NCORES)])
    return (new_states.astype(np.float32), new_e.astype(np.float32),
            new_roles.astype(np.int32), new_pos.astype(np.int32),
            new_org.astype(np.int32))
